# revision 19
# baseline (speedup 1.0000x reference)
"""Trainium2 Bass kernel for nn_ExLoss (memory-bank cross-entropy + momentum
scatter update), sharded over 8 NeuronCores.

Math:
    logits = X @ V.T                      [B, C]
    loss   = -mean_i(logits[i, y_i] - logsumexp_i)
    V_new  = sequential momentum scatter: for i in batch order:
                 V[y_i] = m*V[y_i] + (1-m)*x_i
             closed form per class y with hits i_1<...<i_k:
                 V_new[y] = m^k V[y] + sum_j (1-m) m^(k-1-j) x_{i_j}

Sharding (tensor-parallel over classes): core c owns V rows
[c*2048, (c+1)*2048). Each core computes its logits block X @ Vc.T (f32r
matmuls), partial row-max/sumexp per 1024-wide block (the (m, s) softmax
partials are combined exactly on the host), the target-logit partial for the
samples routed to it, and its V_new shard:
    V_new_c = decay ⊙ Vc + A01^T @ (w ⊙ X_own)

The momentum recurrence reduces to host-side index routing: each routed
sample gets a closed-form weight w = (1-m) m^(#later hits of its class) and
each class a decay m^k. The scatter matmul runs on the tensor engine in f32r
(11-bit mantissa): to keep V_new at fp32 accuracy the one-hot matrix holds
exact 1.0s and the w*x rows are split into f32r-exact hi+lo parts
(wx = wxh + wxl, each 11-bit), so every PE product is exact and only the
fp32 PSUM accumulation rounds — same envelope as the fp32 reference.
"""

import numpy as np

import concourse.bass as bass
import concourse.mybir as mybir
from concourse.bass import IndirectOffsetOnAxis
from concourse.bass_utils import run_bass_kernel_spmd
from concourse.tile import TileContext
from concourse.vector_clock import ScopedClock

# ---------------- problem constants (hardcoded per contract) ----------------
B, C, D = 2048, 16384, 512
MOM = 0.9
NCORES = 8
CSH = C // NCORES  # 2048 classes per core
NOWN = 384  # padded routed-sample slots per core (actual max ~274)
P = 128
NB = 512  # psum bank width (fp32)
NBW = 1024  # logits block width (2 banks)
NBLK = CSH // NBW  # stat blocks per i-tile (2)
NIT = B // P  # 16 batch tiles
NKT = D // P  # 4 contraction chunks
NOT = NOWN // P  # 3 own-sample tiles
NRT = CSH // P  # 16 class tiles
SUB = 4  # row-max subsample stride (subsampled max only shifts exp() scaling;
#          host combine of (m, s) partials is exact regardless)

F32 = mybir.dt.float32
F32R = mybir.dt.float32r
I32 = mybir.dt.int32
BF16 = mybir.dt.bfloat16
ALU = mybir.AluOpType
AXX = mybir.AxisListType.X


# ---------------- walrus workarounds ----------------
# This container's walrus build encodes at most ONE sync-wait per instruction.
# 1) TileContext's kernel-tail drain collects one wait per live semaphore ->
#    patch it to spill waits onto chained NOPs.
# 2) Tile's sem assignment can attach 2 waits to any instruction -> post-pass
#    hoists extras onto same-engine NOPs inserted just before the owner.

_ctr = [0]


def _wait_nop(engine, wait):
    _ctr[0] += 1
    return mybir.InstNoOp(
        name=f"waitsplit-{_ctr[0]}",
        engine=engine,
        ins=[],
        outs=[],
        sync_info=mybir.SyncInfo(on_wait=[wait], on_update=[]),
    )


def _legalize_waits(nc):
    n = 0
    for fn in nc.m.functions:
        for bb in fn.blocks:
            insts = list(bb.instructions)
            if not any(
                i.sync_info is not None and len(i.sync_info.on_wait) > 1
                for i in insts
            ):
                continue
            new = []
            for inst in insts:
                si = inst.sync_info
                if si is not None and len(si.on_wait) > 1:
                    waits = list(si.on_wait)
                    for w in waits[:-1]:
                        new.append(_wait_nop(inst.engine, w))
                    inst.sync_info = mybir.SyncInfo(
                        on_wait=[waits[-1]], on_update=list(si.on_update)
                    )
                    n += 1
                new.append(inst)
            bb.instructions = new
    return n


def _patched_drain_and_barrier(self, tick_clock, wait_clock):
    nc = self.nc
    drain_inst = nc.sync.drain()
    wait_clock.add_sem_waits(
        drain_inst.ins, ScopedClock({None: tick_clock.global_clock})
    )
    si = drain_inst.ins.sync_info
    if si is not None and len(si.on_wait) > 1:
        waits = list(si.on_wait)
        drain_inst.ins.sync_info = mybir.SyncInfo(
            on_wait=waits[:1], on_update=list(si.on_update)
        )
        for w in waits[1:]:
            nop = nc.sync.nop(nofuse=True)
            nop.ins.sync_info = mybir.SyncInfo(on_wait=[w], on_update=[])
    nc.all_engine_barrier()
    assert self.sems is not None
    popped = nc._tile_sem_poison_stack.pop()
    assert popped is self._sem_poison
    nc.clear_and_free_semaphores(list(self.sems.allocated().values()))
    nc.all_engine_barrier()


def _apply_patches():
    import concourse.tile as tile_mod

    tile_mod.TileContext._drain_and_barrier = _patched_drain_and_barrier


# ---------------- kernel builder ----------------
def _build():
    _apply_patches()
    nc = bass.Bass("TRN2")

    # inputs (per-core)
    xt = nc.dram_tensor("xt", [D, B], F32R, kind="ExternalInput")  # X^T
    vt = nc.dram_tensor("vt", [D, CSH], F32R, kind="ExternalInput")  # Vc^T
    vc = nc.dram_tensor("vc", [CSH, D], F32, kind="ExternalInput")  # Vc
    xo = nc.dram_tensor("xo", [NOWN, D], F32, kind="ExternalInput")  # routed X
    yo = nc.dram_tensor("yo", [P, NOT], F32, kind="ExternalInput")  # local cls
    wo = nc.dram_tensor("wo", [P, NOT], F32, kind="ExternalInput")  # weights
    yoi = nc.dram_tensor("yoi", [P, NOT], I32, kind="ExternalInput")
    dc = nc.dram_tensor("dc", [P, NRT], F32, kind="ExternalInput")  # decay

    # outputs
    vout = nc.dram_tensor("vout", [CSH, D], F32, kind="ExternalOutput")
    ms = nc.dram_tensor("ms", [P, NIT * NBLK], F32, kind="ExternalOutput")
    ss = nc.dram_tensor("ss", [P, NIT * NBLK], F32, kind="ExternalOutput")
    ts = nc.dram_tensor("ts", [P, NOT], F32, kind="ExternalOutput")

    with TileContext(nc) as tc:
        with (
            tc.tile_pool(name="big", bufs=1) as big,
            tc.tile_pool(name="scr", bufs=2) as scr,
            tc.tile_pool(name="vnp", bufs=3) as vnp,
            tc.tile_pool(name="gth", bufs=2) as gth,
            tc.tile_pool(name="vcp", bufs=3) as vcp,
            tc.tile_pool(name="psu", bufs=4, space="PSUM") as psu,
        ):
            # ---- device iota [P, CSH] (f32; values < 2048 are exact) ----
            iota_t = big.tile([P, CSH], F32, tag="iota", name="iota_t")
            nc.gpsimd.iota(
                iota_t[:],
                pattern=[[1, CSH]],
                base=0,
                channel_multiplier=0,
                allow_small_or_imprecise_dtypes=True,
            )

            # ---- small loads ----
            yo_t = big.tile([P, NOT], F32, tag="yo", name="yo_t")
            nc.sync.dma_start(out=yo_t[:], in_=yo[:])
            wo_t = big.tile([P, NOT], F32, tag="wo", name="wo_t")
            nc.sync.dma_start(out=wo_t[:], in_=wo[:])
            yoi_t = big.tile([P, NOT], I32, tag="yoi", name="yoi_t")
            nc.sync.dma_start(out=yoi_t[:], in_=yoi[:])
            dc_t = big.tile([P, NRT], F32, tag="dc", name="dc_t")
            nc.sync.dma_start(out=dc_t[:], in_=dc[:])
            xo_t = []
            for t_ in range(NOT):
                t = big.tile([P, D], F32, tag=f"xo{t_}", name=f"xo{t_}")
                nc.sync.dma_start(out=t[:], in_=xo[t_ * P : (t_ + 1) * P, :])
                xo_t.append(t)

            # ---- main loads, range-streamed so logits start early ----
            xt_t = [
                big.tile([P, B], F32R, tag=f"xt{k}", name=f"xt{k}")
                for k in range(NKT)
            ]
            vt_t = [
                big.tile([P, CSH], F32R, tag=f"vt{k}", name=f"vt{k}")
                for k in range(NKT)
            ]
            XQ = B // 4
            for j in range(2):  # block b=0 cols, 512 at a time
                for k in range(NKT):
                    nc.sync.dma_start(
                        out=vt_t[k][:, j * NB : (j + 1) * NB],
                        in_=vt[k * P : (k + 1) * P, j * NB : (j + 1) * NB],
                    )
                if j == 0:
                    for k in range(NKT):
                        nc.sync.dma_start(
                            out=xt_t[k][:, 0:XQ],
                            in_=xt[k * P : (k + 1) * P, 0:XQ],
                        )
            for j in range(2, 4):  # block b=1 cols
                for k in range(NKT):
                    nc.sync.dma_start(
                        out=vt_t[k][:, j * NB : (j + 1) * NB],
                        in_=vt[k * P : (k + 1) * P, j * NB : (j + 1) * NB],
                    )
            for q in range(1, 4):
                for k in range(NKT):
                    nc.sync.dma_start(
                        out=xt_t[k][:, q * XQ : (q + 1) * XQ],
                        in_=xt[k * P : (k + 1) * P, q * XQ : (q + 1) * XQ],
                    )
            vc_t = []
            for r in range(NRT):
                t = vcp.tile([P, D], F32, tag="vc", name=f"vc{r}")
                nc.sync.dma_start(out=t[:], in_=vc[r * P : (r + 1) * P, :])
                vc_t.append(t)

            # stat accumulators
            ms_t = big.tile([P, NIT * NBLK], F32, tag="ms", name="ms_t")
            ss_t = big.tile([P, NIT * NBLK], F32, tag="ss", name="ss_t")
            ts_t = big.tile([P, NOT], F32, tag="ts", name="ts_t")

            # ---- A01 one-hot builds (exact 1.0 entries) ----
            a_t = []
            for t_ in range(NOT):
                a = big.tile([P, CSH], F32R, tag=f"a{t_}", name=f"a{t_}")
                nc.vector.tensor_scalar(
                    out=a[:],
                    in0=iota_t[:],
                    scalar1=yo_t[:, t_ : t_ + 1],
                    scalar2=None,
                    op0=ALU.is_equal,
                )
                a_t.append(a)

            # ---- w*x rows split into f32r-exact hi+lo parts ----
            wxh_t = []
            wxl_t = []
            for t_ in range(NOT):
                wx = scr.tile([P, D], F32, tag="wxf", name=f"wxf{t_}")
                nc.vector.tensor_scalar(
                    out=wx[:],
                    in0=xo_t[t_][:],
                    scalar1=wo_t[:, t_ : t_ + 1],
                    scalar2=None,
                    op0=ALU.mult,
                )
                wxh = big.tile([P, D], F32R, tag=f"wxh{t_}", name=f"wxh{t_}")
                nc.vector.tensor_copy(out=wxh[:], in_=wx[:])  # rounds to f32r
                wxl = big.tile([P, D], F32R, tag=f"wxl{t_}", name=f"wxl{t_}")
                nc.vector.tensor_tensor(
                    out=wxl[:],
                    in0=wx[:],
                    in1=wxh[:].bitcast(F32),
                    op=ALU.subtract,
                )
                wxh_t.append(wxh)
                wxl_t.append(wxl)

            # ---- stage 3: target-logit partials (gather + dot) ----
            for t_ in range(NOT):
                vg = gth.tile([P, D], F32, tag="vg", name=f"vg{t_}")
                nc.gpsimd.indirect_dma_start(
                    out=vg[:],
                    out_offset=None,
                    in_=vc[:],
                    in_offset=IndirectOffsetOnAxis(
                        ap=yoi_t[:, t_ : t_ + 1], axis=0
                    ),
                )
                tt = scr.tile([P, D], F32, tag="tt", name=f"tt{t_}")
                nc.vector.tensor_tensor(
                    out=tt[:], in0=vg[:], in1=xo_t[t_][:], op=ALU.mult
                )
                nc.vector.tensor_reduce(
                    out=ts_t[:, t_ : t_ + 1], in_=tt[:], axis=AXX, op=ALU.add
                )

            # ---- stage 2a: scatter matmuls (2-term), decoupled to SBUF ----
            s_sb = []
            for r in range(NRT):
                ps = psu.tile([P, NB], F32, tag="psu", name=f"ps{r}")
                rsl = slice(r * P, (r + 1) * P)
                n_mm = 2 * NOT
                mm = 0
                for term in (wxh_t, wxl_t):
                    for t_ in range(NOT):
                        nc.tensor.matmul(
                            ps[:],
                            lhsT=a_t[t_][:, rsl],
                            rhs=term[t_][:],
                            start=(mm == 0),
                            stop=(mm == n_mm - 1),
                        )
                        mm += 1
                sb = big.tile([P, D], F32, tag=f"ssb{r}", name=f"ssb{r}")
                if r % 2 == 0:
                    nc.scalar.copy(out=sb[:], in_=ps[:])
                else:
                    nc.vector.tensor_copy(out=sb[:], in_=ps[:])
                s_sb.append(sb)

            # ---- stage 2b: V_new = decay * Vc + S ----
            for r in range(NRT):
                rsl = slice(r * P, (r + 1) * P)
                vn = vnp.tile([P, D], F32, tag="vn", name=f"vn{r}")
                nc.vector.tensor_scalar(
                    out=vn[:],
                    in0=vc_t[r][:],
                    scalar1=dc_t[:, r : r + 1],
                    scalar2=None,
                    op0=ALU.mult,
                )
                nc.vector.tensor_tensor(
                    out=vn[:], in0=vn[:], in1=s_sb[r][:], op=ALU.add
                )
                nc.sync.dma_start(out=vout[rsl, :], in_=vn[:])

            # ---- stage 1: logits + softmax partials, b-outer passes ----
            for b in range(NBLK):
                for i in range(NIT):
                    pt = psu.tile([P, NBW], F32, tag="psu", name=f"pl{i}_{b}")
                    isl = slice(i * P, (i + 1) * P)
                    for k in range(NKT):
                        for j2 in range(NBW // NB):
                            jc = b * NBW + j2 * NB
                            nc.tensor.matmul(
                                pt[:, j2 * NB : (j2 + 1) * NB],
                                lhsT=xt_t[k][:, isl],
                                rhs=vt_t[k][:, jc : jc + NB],
                                start=(k == 0),
                                stop=(k == NKT - 1),
                            )
                    col = i * NBLK + b
                    nc.vector.tensor_reduce(
                        out=ms_t[:, col : col + 1],
                        in_=pt[:, 0:NBW:SUB],
                        axis=AXX,
                        op=ALU.max,
                        negate=True,
                    )
                    eo = scr.tile(
                        [P, NBW], BF16, tag="eo", name=f"eo{i}_{b}", bufs=2
                    )
                    nc.scalar.activation(
                        out=eo[:],
                        in_=pt[:],
                        func=mybir.ActivationFunctionType.Exp,
                        bias=ms_t[:, col : col + 1],
                        scale=1.0,
                        accum_out=ss_t[:, col : col + 1],
                    )

            # ---- store stats ----
            nc.sync.dma_start(out=ms[:], in_=ms_t[:])
            nc.sync.dma_start(out=ss[:], in_=ss_t[:])
            nc.sync.dma_start(out=ts[:], in_=ts_t[:])

    _legalize_waits(nc)
    return nc


_NC = None


def _get_nc():
    global _NC
    if _NC is None:
        _NC = _build()
    return _NC


# ---------------- host-side routing ----------------
def _scatter_weights(Y):
    """Per-sample scatter weight and per-class decay (closed-form momentum)."""
    order = np.argsort(Y, kind="stable")
    ys = Y[order]
    starts = np.r_[0, np.nonzero(np.diff(ys))[0] + 1]
    counts = np.diff(np.r_[starts, len(ys)])
    group_id = np.zeros(len(ys), dtype=np.int64)
    group_id[starts[1:]] = 1
    group_id = np.cumsum(group_id)
    rank_sorted = np.arange(len(ys)) - starts[group_id]  # occurrence rank
    k = counts[group_id]
    w_sorted = (1.0 - MOM) * MOM ** (k - 1 - rank_sorted)
    w = np.zeros(B, dtype=np.float32)
    w[order] = w_sorted.astype(np.float32)
    decay = np.ones(C, dtype=np.float32)
    decay[ys[starts]] = (MOM ** counts).astype(np.float32)
    return w, decay


def _prepare_in_maps(X, Y, V):
    w, decay = _scatter_weights(Y)
    XT = np.ascontiguousarray(X.T)
    in_maps = []
    for c in range(NCORES):
        lo = c * CSH
        Vc = V[lo : lo + CSH]
        own = np.nonzero((Y >= lo) & (Y < lo + CSH))[0]
        n = len(own)
        assert n <= NOWN, f"core {c} owns {n} > {NOWN} samples"
        xo = np.zeros((NOWN, D), dtype=np.float32)
        xo[:n] = X[own]
        yloc = np.zeros(NOWN, dtype=np.int64)  # pads -> row 0 (w=0)
        yloc[:n] = Y[own] - lo
        yof = np.full(NOWN, -1.0, dtype=np.float32)  # pads match nothing
        yof[:n] = yloc[:n].astype(np.float32)
        wvec = np.zeros(NOWN, dtype=np.float32)
        wvec[:n] = w[own]
        in_maps.append(
            {
                "xt": XT,
                "vt": np.ascontiguousarray(Vc.T),
                "vc": np.ascontiguousarray(Vc),
                "xo": xo,
                "yo": np.ascontiguousarray(yof.reshape(NOT, P).T),
                "wo": np.ascontiguousarray(wvec.reshape(NOT, P).T),
                "yoi": np.ascontiguousarray(
                    yloc.reshape(NOT, P).T.astype(np.int32)
                ),
                "dc": np.ascontiguousarray(
                    decay[lo : lo + CSH].reshape(NRT, P).T
                ),
            }
        )
    return in_maps


def _combine(results):
    """Host combine of per-core outputs -> (loss, V_new)."""
    V_new = np.concatenate([r["vout"] for r in results], axis=0)

    # ms[p, i*NBLK+blk] = -max over block (c, blk) for batch row i*128+p
    nm = np.stack([r["ms"] for r in results])  # [8, 128, NIT*NBLK]
    sv = np.stack([r["ss"] for r in results])
    nm = nm.reshape(NCORES, P, NIT, NBLK)
    sv = sv.reshape(NCORES, P, NIT, NBLK)
    # -> [B, 8*NBLK] keyed by batch row b = i*128+p
    m = -np.transpose(nm, (2, 1, 0, 3)).reshape(B, NCORES * NBLK)
    s = np.transpose(sv, (2, 1, 0, 3)).reshape(B, NCORES * NBLK)
    M = m.max(axis=1)
    S = (s * np.exp(m - M[:, None])).sum(axis=1)
    lse = M + np.log(S)

    t_total = sum(float(r["ts"].sum()) for r in results)
    loss = np.float32(-(t_total - float(lse.sum())) / B)
    return loss, V_new


# cached PJRT executable: bass2jax.run_bass_via_pjrt builds a fresh
# jit(shard_map(closure)) per call (seconds of retracing); build it once.
_EXEC = None


def _build_exec(nc):
    import jax
    from jax.experimental.shard_map import shard_map
    from jax.sharding import Mesh, PartitionSpec

    from concourse import bass2jax

    bass2jax.install_neuronx_cc_hook()
    partition_name = (
        nc.partition_id_tensor.name if nc.partition_id_tensor else None
    )
    in_names, out_names, out_avals, zero_shapes = [], [], [], []
    for alloc in nc.m.functions[0].allocations:
        if not isinstance(alloc, mybir.MemoryLocationSet):
            continue
        name = alloc.memorylocations[0].name
        if alloc.kind == "ExternalInput":
            if name != partition_name:
                in_names.append(name)
        elif alloc.kind == "ExternalOutput":
            shape = tuple(alloc.tensor_shape)
            dtype = mybir.dt.np(alloc.dtype)
            out_names.append(name)
            out_avals.append(jax.core.ShapedArray(shape, dtype))
            zero_shapes.append((shape, dtype))
    n_params = len(in_names)
    all_names = list(in_names) + list(out_names)
    if partition_name is not None:
        all_names.append(partition_name)
    donate = tuple(range(n_params, n_params + len(out_names)))

    def _body(*args):
        operands = list(args)
        if partition_name is not None:
            operands.append(bass2jax.partition_id_tensor())
        outs = bass2jax._bass_exec_p.bind(
            *operands,
            out_avals=tuple(out_avals),
            in_names=tuple(all_names),
            out_names=tuple(out_names),
            lowering_input_output_aliases=(),
            sim_require_finite=True,
            sim_require_nnan=True,
            nc=nc,
        )
        return tuple(outs)

    devices = jax.devices()[:NCORES]
    mesh = Mesh(np.asarray(devices), ("core",))
    n_io = n_params + len(out_names)
    sharded = jax.jit(
        shard_map(
            _body,
            mesh=mesh,
            in_specs=(PartitionSpec("core"),) * n_io,
            out_specs=(PartitionSpec("core"),) * len(out_names),
            check_rep=False,
        ),
        donate_argnums=donate,
        keep_unused=True,
    )
    return sharded, in_names, out_names, out_avals, zero_shapes


class _Res:
    def __init__(self, results):
        self.results = results
        self.exec_time_ns = None
        self.mean_exec_time_ns = None
        self.instructions_and_trace = None


def _exec_fast(nc, in_maps):
    global _EXEC
    if _EXEC is None:
        _EXEC = _build_exec(nc)
    sharded, in_names, out_names, out_avals, zero_shapes = _EXEC
    concat_in = [
        np.concatenate([np.asarray(m[name]) for m in in_maps], axis=0)
        for name in in_names
    ]
    concat_zeros = [
        np.zeros((NCORES * s[0], *s[1:]), dt) for s, dt in zero_shapes
    ]
    out_arrs = sharded(*concat_in, *concat_zeros)
    results = [
        {
            name: np.asarray(out_arrs[i]).reshape(
                NCORES, *out_avals[i].shape
            )[c]
            for i, name in enumerate(out_names)
        }
        for c in range(NCORES)
    ]
    return _Res(results)


def run(X, Y, V, trace=False):
    nc = _get_nc()
    in_maps = _prepare_in_maps(X, Y, V)
    try:
        res = _exec_fast(nc, in_maps)
    except Exception:
        res = run_bass_kernel_spmd(
            nc, in_maps, core_ids=list(range(NCORES)), trace=trace
        )
    loss, V_new = _combine(res.results)
    return loss, V_new, res


def kernel(inputs, targets, V):
    X = np.ascontiguousarray(np.asarray(inputs, dtype=np.float32))
    Y = np.asarray(targets).astype(np.int64)
    Vf = np.ascontiguousarray(np.asarray(V, dtype=np.float32))
    loss, V_new, _ = run(X, Y, Vf)
    return loss, V_new


# revision 22
# speedup vs baseline: 1.0252x; 1.0252x over previous
"""Trainium2 Bass kernel for nn_ExLoss (memory-bank cross-entropy + momentum
scatter update), sharded over 8 NeuronCores.

Math:
    logits = X @ V.T                      [B, C]
    loss   = -mean_i(logits[i, y_i] - logsumexp_i)
    V_new  = sequential momentum scatter: for i in batch order:
                 V[y_i] = m*V[y_i] + (1-m)*x_i
             closed form per class y with hits i_1<...<i_k:
                 V_new[y] = m^k V[y] + sum_j (1-m) m^(k-1-j) x_{i_j}

Sharding (tensor-parallel over classes): core c owns V rows
[c*2048, (c+1)*2048). Each core computes its logits block X @ Vc.T (f32r
matmuls), partial row-max/sumexp per 1024-wide block (the (m, s) softmax
partials are combined exactly on the host), the target-logit partial for the
samples routed to it, and its V_new shard:
    V_new_c = decay ⊙ Vc + A01^T @ (w ⊙ X_own)

The momentum recurrence reduces to host-side index routing: each routed
sample gets a closed-form weight w = (1-m) m^(#later hits of its class) and
each class a decay m^k. The scatter matmul runs on the tensor engine in f32r
(11-bit mantissa): to keep V_new at fp32 accuracy the one-hot matrix holds
exact 1.0s and the w*x rows are split into f32r-exact hi+lo parts
(wx = wxh + wxl, each 11-bit), so every PE product is exact and only the
fp32 PSUM accumulation rounds — same envelope as the fp32 reference.
"""

import numpy as np

import concourse.bass as bass
import concourse.mybir as mybir
from concourse.bass import IndirectOffsetOnAxis
from concourse.bass_utils import run_bass_kernel_spmd
from concourse.tile import TileContext
from concourse.vector_clock import ScopedClock

# ---------------- problem constants (hardcoded per contract) ----------------
B, C, D = 2048, 16384, 512
MOM = 0.9
NCORES = 8
CSH = C // NCORES  # 2048 classes per core
NOWN = 384  # padded routed-sample slots per core (actual max ~274)
P = 128
NB = 512  # psum bank width (fp32)
NBW = 1024  # logits block width (2 banks)
NBLK = CSH // NBW  # stat blocks per i-tile (2)
NIT = B // P  # 16 batch tiles
NKT = D // P  # 4 contraction chunks
NOT = NOWN // P  # 3 own-sample tiles
NRT = CSH // P  # 16 class tiles
SUB = 4  # row-max subsample stride (subsampled max only shifts exp() scaling;
#          host combine of (m, s) partials is exact regardless)

F32 = mybir.dt.float32
F32R = mybir.dt.float32r
I32 = mybir.dt.int32
BF16 = mybir.dt.bfloat16
ALU = mybir.AluOpType
AXX = mybir.AxisListType.X


# ---------------- walrus workarounds ----------------
# This container's walrus build encodes at most ONE sync-wait per instruction.
# 1) TileContext's kernel-tail drain collects one wait per live semaphore ->
#    patch it to spill waits onto chained NOPs.
# 2) Tile's sem assignment can attach 2 waits to any instruction -> post-pass
#    hoists extras onto same-engine NOPs inserted just before the owner.

_ctr = [0]


def _wait_nop(engine, wait):
    _ctr[0] += 1
    return mybir.InstNoOp(
        name=f"waitsplit-{_ctr[0]}",
        engine=engine,
        ins=[],
        outs=[],
        sync_info=mybir.SyncInfo(on_wait=[wait], on_update=[]),
    )


def _legalize_waits(nc):
    n = 0
    for fn in nc.m.functions:
        for bb in fn.blocks:
            insts = list(bb.instructions)
            if not any(
                i.sync_info is not None and len(i.sync_info.on_wait) > 1
                for i in insts
            ):
                continue
            new = []
            for inst in insts:
                si = inst.sync_info
                if si is not None and len(si.on_wait) > 1:
                    waits = list(si.on_wait)
                    for w in waits[:-1]:
                        new.append(_wait_nop(inst.engine, w))
                    inst.sync_info = mybir.SyncInfo(
                        on_wait=[waits[-1]], on_update=list(si.on_update)
                    )
                    n += 1
                new.append(inst)
            bb.instructions = new
    return n


def _patched_drain_and_barrier(self, tick_clock, wait_clock):
    nc = self.nc
    drain_inst = nc.sync.drain()
    wait_clock.add_sem_waits(
        drain_inst.ins, ScopedClock({None: tick_clock.global_clock})
    )
    si = drain_inst.ins.sync_info
    if si is not None and len(si.on_wait) > 1:
        waits = list(si.on_wait)
        drain_inst.ins.sync_info = mybir.SyncInfo(
            on_wait=waits[:1], on_update=list(si.on_update)
        )
        for w in waits[1:]:
            nop = nc.sync.nop(nofuse=True)
            nop.ins.sync_info = mybir.SyncInfo(on_wait=[w], on_update=[])
    nc.all_engine_barrier()
    assert self.sems is not None
    popped = nc._tile_sem_poison_stack.pop()
    assert popped is self._sem_poison
    nc.clear_and_free_semaphores(list(self.sems.allocated().values()))
    nc.all_engine_barrier()


def _apply_patches():
    import concourse.tile as tile_mod

    tile_mod.TileContext._drain_and_barrier = _patched_drain_and_barrier


# ---------------- kernel builder ----------------
def _build():
    _apply_patches()
    nc = bass.Bass("TRN2")

    # inputs (per-core)
    xt = nc.dram_tensor("xt", [D, B], F32R, kind="ExternalInput")  # X^T
    vt = nc.dram_tensor("vt", [D, CSH], F32R, kind="ExternalInput")  # Vc^T
    vc = nc.dram_tensor("vc", [CSH, D], F32, kind="ExternalInput")  # Vc
    xo = nc.dram_tensor("xo", [NOWN, D], F32, kind="ExternalInput")  # routed X
    yo = nc.dram_tensor("yo", [P, NOT], F32, kind="ExternalInput")  # local cls
    wo = nc.dram_tensor("wo", [P, NOT], F32, kind="ExternalInput")  # weights
    yoi = nc.dram_tensor("yoi", [P, NOT], I32, kind="ExternalInput")
    dc = nc.dram_tensor("dc", [P, NRT], F32, kind="ExternalInput")  # decay

    # outputs
    vout = nc.dram_tensor("vout", [CSH, D], F32, kind="ExternalOutput")
    ms = nc.dram_tensor("ms", [P, NIT * NBLK], F32, kind="ExternalOutput")
    ss = nc.dram_tensor("ss", [P, NIT * NBLK], F32, kind="ExternalOutput")
    ts = nc.dram_tensor("ts", [P, NOT], F32, kind="ExternalOutput")

    with TileContext(nc) as tc:
        with (
            tc.tile_pool(name="big", bufs=1) as big,
            tc.tile_pool(name="scr", bufs=2) as scr,
            tc.tile_pool(name="vnp", bufs=3) as vnp,
            tc.tile_pool(name="gth", bufs=2) as gth,
            tc.tile_pool(name="vcp", bufs=3) as vcp,
            tc.tile_pool(name="psu", bufs=4, space="PSUM") as psu,
        ):
            # ---- device iota [P, CSH] (f32; values < 2048 are exact),
            # generated in halves so the first A01 columns are ready sooner
            iota_t = big.tile([P, CSH], F32, tag="iota", name="iota_t")
            HW_ = CSH // 2
            for h in range(2):
                nc.gpsimd.iota(
                    iota_t[:, h * HW_ : (h + 1) * HW_],
                    pattern=[[1, HW_]],
                    base=h * HW_,
                    channel_multiplier=0,
                    allow_small_or_imprecise_dtypes=True,
                )

            # ---- small loads (yo/wo/xo0 first: they gate the first
            # scatter matmul; yoi/dc feed later stages) ----
            yo_t = big.tile([P, NOT], F32, tag="yo", name="yo_t")
            nc.sync.dma_start(out=yo_t[:], in_=yo[:])
            wo_t = big.tile([P, NOT], F32, tag="wo", name="wo_t")
            nc.sync.dma_start(out=wo_t[:], in_=wo[:])
            xo_t = []
            for t_ in range(NOT):
                t = big.tile([P, D], F32, tag=f"xo{t_}", name=f"xo{t_}")
                xo_t.append(t)
            nc.sync.dma_start(out=xo_t[0][:], in_=xo[0:P, :])
            yoi_t = big.tile([P, NOT], I32, tag="yoi", name="yoi_t")
            nc.sync.dma_start(out=yoi_t[:], in_=yoi[:])
            dc_t = big.tile([P, NRT], F32, tag="dc", name="dc_t")
            nc.sync.dma_start(out=dc_t[:], in_=dc[:])
            for t_ in range(1, NOT):
                nc.sync.dma_start(
                    out=xo_t[t_][:], in_=xo[t_ * P : (t_ + 1) * P, :]
                )

            # ---- main loads, range-streamed so logits start early ----
            xt_t = [
                big.tile([P, B], F32R, tag=f"xt{k}", name=f"xt{k}")
                for k in range(NKT)
            ]
            vt_t = [
                big.tile([P, CSH], F32R, tag=f"vt{k}", name=f"vt{k}")
                for k in range(NKT)
            ]
            XQ = B // 4
            for j in range(2):  # block b=0 cols, 512 at a time
                for k in range(NKT):
                    nc.sync.dma_start(
                        out=vt_t[k][:, j * NB : (j + 1) * NB],
                        in_=vt[k * P : (k + 1) * P, j * NB : (j + 1) * NB],
                    )
                if j == 0:
                    for k in range(NKT):
                        nc.sync.dma_start(
                            out=xt_t[k][:, 0:XQ],
                            in_=xt[k * P : (k + 1) * P, 0:XQ],
                        )
            for j in range(2, 4):  # block b=1 cols
                for k in range(NKT):
                    nc.sync.dma_start(
                        out=vt_t[k][:, j * NB : (j + 1) * NB],
                        in_=vt[k * P : (k + 1) * P, j * NB : (j + 1) * NB],
                    )
            for q in range(1, 4):
                for k in range(NKT):
                    nc.sync.dma_start(
                        out=xt_t[k][:, q * XQ : (q + 1) * XQ],
                        in_=xt[k * P : (k + 1) * P, q * XQ : (q + 1) * XQ],
                    )
            vc_t = []
            for r in range(NRT):
                t = vcp.tile([P, D], F32, tag="vc", name=f"vc{r}")
                nc.sync.dma_start(out=t[:], in_=vc[r * P : (r + 1) * P, :])
                vc_t.append(t)

            # stat accumulators
            ms_t = big.tile([P, NIT * NBLK], F32, tag="ms", name="ms_t")
            ss_t = big.tile([P, NIT * NBLK], F32, tag="ss", name="ss_t")
            ts_t = big.tile([P, NOT], F32, tag="ts", name="ts_t")

            # ---- A01 one-hot + w*x hi/lo splits. Build order matters for
            # the PE start: first the low column halves of all A01 tiles
            # (feeding scatter groups r<8), then the wx splits, then the
            # high halves.
            a_t = [
                big.tile([P, CSH], F32R, tag=f"a{t_}", name=f"a{t_}")
                for t_ in range(NOT)
            ]
            for h in range(2):
                for t_ in range(NOT):
                    nc.vector.tensor_scalar(
                        out=a_t[t_][:, h * HW_ : (h + 1) * HW_],
                        in0=iota_t[:, h * HW_ : (h + 1) * HW_],
                        scalar1=yo_t[:, t_ : t_ + 1],
                        scalar2=None,
                        op0=ALU.is_equal,
                    )
                if h == 1:
                    continue
                wxh_t = []
                wxl_t = []
                for t_ in range(NOT):
                    wx = scr.tile([P, D], F32, tag="wxf", name=f"wxf{t_}")
                    nc.vector.tensor_scalar(
                        out=wx[:],
                        in0=xo_t[t_][:],
                        scalar1=wo_t[:, t_ : t_ + 1],
                        scalar2=None,
                        op0=ALU.mult,
                    )
                    wxh = big.tile(
                        [P, D], F32R, tag=f"wxh{t_}", name=f"wxh{t_}"
                    )
                    nc.vector.tensor_copy(out=wxh[:], in_=wx[:])  # f32r round
                    wxl = big.tile(
                        [P, D], F32R, tag=f"wxl{t_}", name=f"wxl{t_}"
                    )
                    nc.vector.tensor_tensor(
                        out=wxl[:],
                        in0=wx[:],
                        in1=wxh[:].bitcast(F32),
                        op=ALU.subtract,
                    )
                    wxh_t.append(wxh)
                    wxl_t.append(wxl)

            # ---- stage 3: target-logit partials (gather + dot) ----
            for t_ in range(NOT):
                vg = gth.tile([P, D], F32, tag="vg", name=f"vg{t_}")
                nc.gpsimd.indirect_dma_start(
                    out=vg[:],
                    out_offset=None,
                    in_=vc[:],
                    in_offset=IndirectOffsetOnAxis(
                        ap=yoi_t[:, t_ : t_ + 1], axis=0
                    ),
                )
                tt = scr.tile([P, D], F32, tag="tt", name=f"tt{t_}")
                nc.vector.tensor_tensor(
                    out=tt[:], in0=vg[:], in1=xo_t[t_][:], op=ALU.mult
                )
                nc.vector.tensor_reduce(
                    out=ts_t[:, t_ : t_ + 1], in_=tt[:], axis=AXX, op=ALU.add
                )

            # ---- stage 2a: scatter matmuls (2-term), decoupled to SBUF ----
            s_sb = []
            for r in range(NRT):
                ps = psu.tile([P, NB], F32, tag="psu", name=f"ps{r}")
                rsl = slice(r * P, (r + 1) * P)
                n_mm = 2 * NOT
                mm = 0
                for t_ in range(NOT):
                    for term in (wxh_t, wxl_t):
                        nc.tensor.matmul(
                            ps[:],
                            lhsT=a_t[t_][:, rsl],
                            rhs=term[t_][:],
                            start=(mm == 0),
                            stop=(mm == n_mm - 1),
                        )
                        mm += 1
                sb = big.tile([P, D], F32, tag=f"ssb{r}", name=f"ssb{r}")
                if r % 2 == 0:
                    nc.scalar.copy(out=sb[:], in_=ps[:])
                else:
                    nc.vector.tensor_copy(out=sb[:], in_=ps[:])
                s_sb.append(sb)

            # ---- stage 2b: V_new = decay * Vc + S ----
            for r in range(NRT):
                rsl = slice(r * P, (r + 1) * P)
                vn = vnp.tile([P, D], F32, tag="vn", name=f"vn{r}")
                nc.vector.tensor_scalar(
                    out=vn[:],
                    in0=vc_t[r][:],
                    scalar1=dc_t[:, r : r + 1],
                    scalar2=None,
                    op0=ALU.mult,
                )
                nc.vector.tensor_tensor(
                    out=vn[:], in0=vn[:], in1=s_sb[r][:], op=ALU.add
                )
                nc.sync.dma_start(out=vout[rsl, :], in_=vn[:])

            # ---- stage 1: logits + softmax partials, b-outer passes ----
            for b in range(NBLK):
                for i in range(NIT):
                    pt = psu.tile([P, NBW], F32, tag="psu", name=f"pl{i}_{b}")
                    isl = slice(i * P, (i + 1) * P)
                    for k in range(NKT):
                        for j2 in range(NBW // NB):
                            jc = b * NBW + j2 * NB
                            nc.tensor.matmul(
                                pt[:, j2 * NB : (j2 + 1) * NB],
                                lhsT=xt_t[k][:, isl],
                                rhs=vt_t[k][:, jc : jc + NB],
                                start=(k == 0),
                                stop=(k == NKT - 1),
                            )
                    col = i * NBLK + b
                    nc.vector.tensor_reduce(
                        out=ms_t[:, col : col + 1],
                        in_=pt[:, 0:NBW:SUB],
                        axis=AXX,
                        op=ALU.max,
                        negate=True,
                    )
                    eo = scr.tile(
                        [P, NBW], BF16, tag="eo", name=f"eo{i}_{b}", bufs=2
                    )
                    nc.scalar.activation(
                        out=eo[:],
                        in_=pt[:],
                        func=mybir.ActivationFunctionType.Exp,
                        bias=ms_t[:, col : col + 1],
                        scale=1.0,
                        accum_out=ss_t[:, col : col + 1],
                    )

            # ---- store stats ----
            nc.sync.dma_start(out=ms[:], in_=ms_t[:])
            nc.sync.dma_start(out=ss[:], in_=ss_t[:])
            nc.sync.dma_start(out=ts[:], in_=ts_t[:])

    _legalize_waits(nc)
    return nc


_NC = None


def _get_nc():
    global _NC
    if _NC is None:
        _NC = _build()
    return _NC


# ---------------- host-side routing ----------------
def _scatter_weights(Y):
    """Per-sample scatter weight and per-class decay (closed-form momentum)."""
    order = np.argsort(Y, kind="stable")
    ys = Y[order]
    starts = np.r_[0, np.nonzero(np.diff(ys))[0] + 1]
    counts = np.diff(np.r_[starts, len(ys)])
    group_id = np.zeros(len(ys), dtype=np.int64)
    group_id[starts[1:]] = 1
    group_id = np.cumsum(group_id)
    rank_sorted = np.arange(len(ys)) - starts[group_id]  # occurrence rank
    k = counts[group_id]
    w_sorted = (1.0 - MOM) * MOM ** (k - 1 - rank_sorted)
    w = np.zeros(B, dtype=np.float32)
    w[order] = w_sorted.astype(np.float32)
    decay = np.ones(C, dtype=np.float32)
    decay[ys[starts]] = (MOM ** counts).astype(np.float32)
    return w, decay


def _prepare_in_maps(X, Y, V):
    w, decay = _scatter_weights(Y)
    XT = np.ascontiguousarray(X.T)
    in_maps = []
    for c in range(NCORES):
        lo = c * CSH
        Vc = V[lo : lo + CSH]
        own = np.nonzero((Y >= lo) & (Y < lo + CSH))[0]
        n = len(own)
        assert n <= NOWN, f"core {c} owns {n} > {NOWN} samples"
        xo = np.zeros((NOWN, D), dtype=np.float32)
        xo[:n] = X[own]
        yloc = np.zeros(NOWN, dtype=np.int64)  # pads -> row 0 (w=0)
        yloc[:n] = Y[own] - lo
        yof = np.full(NOWN, -1.0, dtype=np.float32)  # pads match nothing
        yof[:n] = yloc[:n].astype(np.float32)
        wvec = np.zeros(NOWN, dtype=np.float32)
        wvec[:n] = w[own]
        in_maps.append(
            {
                "xt": XT,
                "vt": np.ascontiguousarray(Vc.T),
                "vc": np.ascontiguousarray(Vc),
                "xo": xo,
                "yo": np.ascontiguousarray(yof.reshape(NOT, P).T),
                "wo": np.ascontiguousarray(wvec.reshape(NOT, P).T),
                "yoi": np.ascontiguousarray(
                    yloc.reshape(NOT, P).T.astype(np.int32)
                ),
                "dc": np.ascontiguousarray(
                    decay[lo : lo + CSH].reshape(NRT, P).T
                ),
            }
        )
    return in_maps


def _combine(results):
    """Host combine of per-core outputs -> (loss, V_new)."""
    V_new = np.concatenate([r["vout"] for r in results], axis=0)

    # ms[p, i*NBLK+blk] = -max over block (c, blk) for batch row i*128+p
    nm = np.stack([r["ms"] for r in results])  # [8, 128, NIT*NBLK]
    sv = np.stack([r["ss"] for r in results])
    nm = nm.reshape(NCORES, P, NIT, NBLK)
    sv = sv.reshape(NCORES, P, NIT, NBLK)
    # -> [B, 8*NBLK] keyed by batch row b = i*128+p
    m = -np.transpose(nm, (2, 1, 0, 3)).reshape(B, NCORES * NBLK)
    s = np.transpose(sv, (2, 1, 0, 3)).reshape(B, NCORES * NBLK)
    M = m.max(axis=1)
    S = (s * np.exp(m - M[:, None])).sum(axis=1)
    lse = M + np.log(S)

    t_total = sum(float(r["ts"].sum()) for r in results)
    loss = np.float32(-(t_total - float(lse.sum())) / B)
    return loss, V_new


# cached PJRT executable: bass2jax.run_bass_via_pjrt builds a fresh
# jit(shard_map(closure)) per call (seconds of retracing); build it once.
_EXEC = None


def _build_exec(nc):
    import jax
    from jax.experimental.shard_map import shard_map
    from jax.sharding import Mesh, PartitionSpec

    from concourse import bass2jax

    bass2jax.install_neuronx_cc_hook()
    partition_name = (
        nc.partition_id_tensor.name if nc.partition_id_tensor else None
    )
    in_names, out_names, out_avals, zero_shapes = [], [], [], []
    for alloc in nc.m.functions[0].allocations:
        if not isinstance(alloc, mybir.MemoryLocationSet):
            continue
        name = alloc.memorylocations[0].name
        if alloc.kind == "ExternalInput":
            if name != partition_name:
                in_names.append(name)
        elif alloc.kind == "ExternalOutput":
            shape = tuple(alloc.tensor_shape)
            dtype = mybir.dt.np(alloc.dtype)
            out_names.append(name)
            out_avals.append(jax.core.ShapedArray(shape, dtype))
            zero_shapes.append((shape, dtype))
    n_params = len(in_names)
    all_names = list(in_names) + list(out_names)
    if partition_name is not None:
        all_names.append(partition_name)
    donate = tuple(range(n_params, n_params + len(out_names)))

    def _body(*args):
        operands = list(args)
        if partition_name is not None:
            operands.append(bass2jax.partition_id_tensor())
        outs = bass2jax._bass_exec_p.bind(
            *operands,
            out_avals=tuple(out_avals),
            in_names=tuple(all_names),
            out_names=tuple(out_names),
            lowering_input_output_aliases=(),
            sim_require_finite=True,
            sim_require_nnan=True,
            nc=nc,
        )
        return tuple(outs)

    devices = jax.devices()[:NCORES]
    mesh = Mesh(np.asarray(devices), ("core",))
    n_io = n_params + len(out_names)
    sharded = jax.jit(
        shard_map(
            _body,
            mesh=mesh,
            in_specs=(PartitionSpec("core"),) * n_io,
            out_specs=(PartitionSpec("core"),) * len(out_names),
            check_rep=False,
        ),
        donate_argnums=donate,
        keep_unused=True,
    )
    return sharded, in_names, out_names, out_avals, zero_shapes


class _Res:
    def __init__(self, results):
        self.results = results
        self.exec_time_ns = None
        self.mean_exec_time_ns = None
        self.instructions_and_trace = None


def _exec_fast(nc, in_maps):
    global _EXEC
    if _EXEC is None:
        _EXEC = _build_exec(nc)
    sharded, in_names, out_names, out_avals, zero_shapes = _EXEC
    concat_in = [
        np.concatenate([np.asarray(m[name]) for m in in_maps], axis=0)
        for name in in_names
    ]
    concat_zeros = [
        np.zeros((NCORES * s[0], *s[1:]), dt) for s, dt in zero_shapes
    ]
    out_arrs = sharded(*concat_in, *concat_zeros)
    results = [
        {
            name: np.asarray(out_arrs[i]).reshape(
                NCORES, *out_avals[i].shape
            )[c]
            for i, name in enumerate(out_names)
        }
        for c in range(NCORES)
    ]
    return _Res(results)


def run(X, Y, V, trace=False):
    nc = _get_nc()
    in_maps = _prepare_in_maps(X, Y, V)
    try:
        res = _exec_fast(nc, in_maps)
    except Exception:
        res = run_bass_kernel_spmd(
            nc, in_maps, core_ids=list(range(NCORES)), trace=trace
        )
    loss, V_new = _combine(res.results)
    return loss, V_new, res


def kernel(inputs, targets, V):
    X = np.ascontiguousarray(np.asarray(inputs, dtype=np.float32))
    Y = np.asarray(targets).astype(np.int64)
    Vf = np.ascontiguousarray(np.asarray(V, dtype=np.float32))
    loss, V_new, _ = run(X, Y, Vf)
    return loss, V_new
